# revision 19
# baseline (speedup 1.0000x reference)
"""Trainium2 Bass kernel for single-head attention (nn_Attention_49881750176038).

reference computes, per batch element b (B=8, S=2048, Dm=1024, Dk=512):
    q = query @ Wq ; k = key @ Wk ; v = value @ Wv          # [S, Dk]
    pre = (q @ k.T) / sqrt(Dk) + mask * (-1e9)              # [S, S]
    score = softmax(pre, axis=-1)
    out = score @ v                                          # [S, Dk]
returns (out, score).

Sharding: pure data-parallel over batch — core c owns batch element c.

Per-core plan (all matmuls fp16 operands, fp32 PSUM accumulation — fp16
products are exact in fp32, so matmul error is just the fp16 input
quantization ~2.4e-4):
  phase 1: DMA-transpose-load q^T/k^T/v^T (fp16, host-cast), project:
      Q^T[dk, q], K^T[dk, s] (stationary = W tiles), V[s, dk]
      (stationary = v^T tiles).
  phase 2 (per 128-row q-tile):
      S chunks in PSUM via Q^T.T @ K^T; mask folded in via an
      identity-matmul add (host pre-scales mask by -60000 in fp16;
      -60000/sqrt(512) drives exp to exactly 0, matching the reference's
      -1e9 additive mask for binary masks);
      exp on ScalarE with scale=1/sqrt(Dk), accum_out = row-sum (free
      softmax denominator); reciprocal on VectorE;
      score rows = exp * recip -> fp32 -> HBM;
      DMA-transpose exp(S) row-tile (SBUF->SBUF, 3D dest) for the
      P @ V matmul; normalize out rows by recip -> HBM.
No collectives needed.
"""
import sys

sys.path.insert(0, "/opt/trn_rl_repo")

import numpy as np
from contextlib import ExitStack

import concourse.bacc as bacc
import concourse.tile as tile
from concourse import mybir
from concourse.bass_utils import run_bass_kernel_spmd

B, S, DM, DK, P = 8, 2048, 1024, 512, 128
NT_S = S // P      # 16 q/s tiles
NT_DM = DM // P    # 8 dm tiles
NT_DK = DK // P    # 4 dk tiles
NCH = S // 512     # 4 s-chunks of 512
CH = 512
SCALE = 1.0 / float(np.sqrt(np.float32(DK)))
MASKC = -60000.0   # fp16-representable; * SCALE -> -2652 -> exp -> 0.0

f16 = mybir.dt.float16
f32 = mybir.dt.float32

_NC = None


def _build():
    nc = bacc.Bacc("TRN2", target_bir_lowering=False, debug=False, num_devices=B)

    q16 = nc.dram_tensor("q16", [S, DM], f16, kind="ExternalInput").ap()
    k16 = nc.dram_tensor("k16", [S, DM], f16, kind="ExternalInput").ap()
    v16 = nc.dram_tensor("v16", [S, DM], f16, kind="ExternalInput").ap()
    m16 = nc.dram_tensor("m16", [S, S], f16, kind="ExternalInput").ap()
    wq16 = nc.dram_tensor("wq16", [DM, DK], f16, kind="ExternalInput").ap()
    wk16 = nc.dram_tensor("wk16", [DM, DK], f16, kind="ExternalInput").ap()
    wv16 = nc.dram_tensor("wv16", [DM, DK], f16, kind="ExternalInput").ap()

    score_o = nc.dram_tensor("score_o", [S, S], f32, kind="ExternalOutput").ap()
    out_o = nc.dram_tensor("out_o", [S, DK], f32, kind="ExternalOutput").ap()

    ident_np = np.eye(P, dtype=np.float16)
    ident_dram = nc.inline_tensor(ident_np, name="ident").ap()

    with tile.TileContext(nc) as tc:
        with ExitStack() as ctx:
            consts = ctx.enter_context(tc.tile_pool(name="consts", bufs=1))
            ps_pool = ctx.enter_context(tc.tile_pool(name="ps", bufs=7, space="PSUM"))
            pv_pool = ctx.enter_context(tc.tile_pool(name="pv", bufs=1, space="PSUM"))

            ident = consts.tile([P, P], f16)
            nc.sync.dma_start(out=ident, in_=ident_dram)
            # HAM warmup: keep the PE busy during the initial transpose fill
            # so the clock gate opens before real matmuls arrive.
            warm = ps_pool.tile([P, CH], f32, tag="ps")
            for i in range(48):
                nc.tensor.matmul(
                    warm[:, 0:P], ident, ident, start=(i == 0), stop=(i == 47)
                )
            warm_sink = consts.tile([P, 1], f32)
            nc.scalar.copy(out=warm_sink, in_=warm[:, 0:1])
            QT = consts.tile([P, NT_DK, S], f16)  # Q^T: [dk_in_tile, dk_tile, q]
            KT = consts.tile([P, NT_DK, S], f16)
            V = consts.tile([P, NT_S, DK], f16)   # V: [s_in_tile, s_tile, dk]

            # ---- phase 1: transposed loads + projections (scoped pools,
            # freed before phase 2 opens its pools)
            with ExitStack() as p1:
                w_pool = p1.enter_context(tc.tile_pool(name="w", bufs=1))
                xt_pool = p1.enter_context(tc.tile_pool(name="xt", bufs=18))

                wq_sb = w_pool.tile([P, NT_DM, DK], f16)
                wk_sb = w_pool.tile([P, NT_DM, DK], f16)
                wv_sb = w_pool.tile([P, NT_DM, DK], f16)
                for wsb, wap in ((wq_sb, wq16), (wk_sb, wk16), (wv_sb, wv16)):
                    nc.sync.dma_start(
                        out=wsb, in_=wap.rearrange("(t p) d -> p t d", p=P)
                    )

                def load_xT(xap):
                    ts = []
                    for ci in range(NT_DM):
                        t = xt_pool.tile([P, S], f16, tag="xt")
                        nc.sync.dma_start(
                            out=t, in_=xap[:, ci * P : (ci + 1) * P], transpose=True
                        )
                        ts.append(t)
                    return ts

                def project(xts, wsb, dst):
                    for qc in range(NCH):
                        for dki in range(NT_DK):
                            ps = ps_pool.tile([P, CH], f32, tag="ps")
                            for dmi in range(NT_DM):
                                nc.tensor.matmul(
                                    ps,
                                    wsb[:, dmi, dki * P : (dki + 1) * P],
                                    xts[dmi][:, qc * CH : (qc + 1) * CH],
                                    start=(dmi == 0),
                                    stop=(dmi == NT_DM - 1),
                                )
                            nc.scalar.copy(
                                out=dst[:, dki, qc * CH : (qc + 1) * CH], in_=ps
                            )

                # K first, in half-column transposes: first proj chunk
                # needs only the 8 first halves -> shorter pipeline fill
                kts = []
                for ci in range(NT_DM):
                    kt_t = xt_pool.tile([P, S], f16, tag="xt", name=f"kt_{ci}")
                    kts.append(kt_t)
                H = S // 2
                for h in range(2):
                    for ci in range(NT_DM):
                        nc.sync.dma_start(
                            out=kts[ci][:, h * H : (h + 1) * H],
                            in_=k16[h * H : (h + 1) * H, ci * P : (ci + 1) * P],
                            transpose=True,
                        )
                qts = load_xT(q16)
                project(kts, wk_sb, KT)
                vts = load_xT(v16)
                project(qts, wq_sb, QT)
                for si in range(NT_S):
                    ps = ps_pool.tile([P, DK], f32, tag="ps")
                    for dmi in range(NT_DM):
                        nc.tensor.matmul(
                            ps,
                            vts[dmi][:, si * P : (si + 1) * P],
                            wv_sb[:, dmi, :],
                            start=(dmi == 0),
                            stop=(dmi == NT_DM - 1),
                        )
                    nc.vector.tensor_copy(V[:, si, :], ps)

            # ---- phase 2 pools (open after phase-1 SBUF is released)
            mask_pool = ctx.enter_context(tc.tile_pool(name="mask", bufs=6))
            exp_pool = ctx.enter_context(tc.tile_pool(name="exp", bufs=4))
            expt_pool = ctx.enter_context(tc.tile_pool(name="expt", bufs=6))
            score_pool = ctx.enter_context(tc.tile_pool(name="score", bufs=3))
            outs_pool = ctx.enter_context(tc.tile_pool(name="outs", bufs=3))
            den_pool = ctx.enter_context(tc.tile_pool(name="den", bufs=6))

            # ---- phase 2: attention per q-tile.
            # P@V is software-pipelined two iterations behind S so the PE
            # never waits on the exp -> transpose chain (and stays HAM-warm).
            def emit_pv(ett, rec, qi):
                po = pv_pool.tile([P, DK], f32, tag="po")
                for si in range(NT_S):
                    nc.tensor.matmul(
                        po,
                        ett[:, si, :],
                        V[:, si, :],
                        start=(si == 0),
                        stop=(si == NT_S - 1),
                    )
                oS = outs_pool.tile([P, DK], f32)
                nc.vector.tensor_scalar_mul(oS, po, rec)
                nc.gpsimd.dma_start(out=out_o[qi * P : (qi + 1) * P, :], in_=oS)

            # mask prefetch (distance 3) so loads never sit behind
            # compute-dependent stores in the ring FIFO
            masks = {}

            def load_mask(qj):
                mt = mask_pool.tile([P, S], f16)
                nc.sync.dma_start(out=mt, in_=m16[qj * P : (qj + 1) * P, :])
                masks[qj] = mt

            for qj in range(3):
                load_mask(qj)

            pending = []
            for qi in range(NT_S):
                if qi + 3 < NT_S:
                    load_mask(qi + 3)
                mt = masks.pop(qi)

                et = exp_pool.tile([P, S], f16)
                den4 = den_pool.tile([P, NCH], f32, tag="den4")
                for sc in range(NCH):
                    ps = ps_pool.tile([P, CH], f32, tag="ps")
                    for dki in range(NT_DK):
                        nc.tensor.matmul(
                            ps,
                            QT[:, dki, qi * P : (qi + 1) * P],
                            KT[:, dki, sc * CH : (sc + 1) * CH],
                            start=(dki == 0),
                            stop=(dki == NT_DK - 1),
                        )
                    nc.vector.tensor_add(ps, ps, mt[:, sc * CH : (sc + 1) * CH])
                    nc.scalar.activation(
                        out=et[:, sc * CH : (sc + 1) * CH],
                        in_=ps,
                        func=mybir.ActivationFunctionType.Exp,
                        scale=SCALE,
                        accum_out=den4[:, sc : sc + 1],
                    )

                den = den_pool.tile([P, 1], f32, tag="den")
                nc.vector.reduce_sum(den, den4, axis=mybir.AxisListType.X)
                rec = den_pool.tile([P, 1], f32, tag="rec")
                nc.vector.reciprocal(rec, den)

                ett = expt_pool.tile([P, NT_S, P], f16)
                nc.sync.dma_start(out=ett, in_=et, transpose=True)

                sc32 = score_pool.tile([P, S], f32)
                nc.vector.tensor_scalar_mul(sc32, et, rec)
                nc.gpsimd.dma_start(
                    out=score_o[qi * P : (qi + 1) * P, :], in_=sc32
                )

                pending.append((ett, rec, qi))
                lag = 3 if qi < NT_S - 3 else NT_S - 1 - qi
                while len(pending) > lag:
                    emit_pv(*pending.pop(0))
            for args in pending:
                emit_pv(*args)

    nc.compile()
    return nc


def _get_nc():
    global _NC
    if _NC is None:
        _NC = _build()
    return _NC


def _make_in_maps(query, key, value, mask, Wq, Wk, Wv):
    query = np.asarray(query, dtype=np.float32)
    key = np.asarray(key, dtype=np.float32)
    value = np.asarray(value, dtype=np.float32)
    mask = np.asarray(mask, dtype=np.float32)
    wq = np.asarray(Wq, dtype=np.float32).astype(np.float16)
    wk = np.asarray(Wk, dtype=np.float32).astype(np.float16)
    wv = np.asarray(Wv, dtype=np.float32).astype(np.float16)
    in_maps = []
    for b in range(B):
        in_maps.append(
            {
                "q16": query[b].astype(np.float16),
                "k16": key[b].astype(np.float16),
                "v16": value[b].astype(np.float16),
                "m16": (mask[b] * np.float32(MASKC)).astype(np.float16),
                "wq16": wq,
                "wk16": wk,
                "wv16": wv,
            }
        )
    return in_maps


def _run(in_maps, trace=False, trace_kwargs=None):
    nc = _get_nc()
    return run_bass_kernel_spmd(
        nc,
        in_maps,
        list(range(B)),
        trace=trace,
        **(trace_kwargs or {}),
    )


def kernel(query, key, value, mask, Wq, Wk, Wv):
    res = _run(_make_in_maps(query, key, value, mask, Wq, Wk, Wv))
    out = np.stack([res.results[b]["out_o"] for b in range(B)])
    score = np.stack([res.results[b]["score_o"] for b in range(B)])
    return out, score


def kernel_traced(query, key, value, mask, Wq, Wk, Wv):
    """Same as kernel() but with NTFF profiling; returns ((out, score), results)."""
    res = _run(_make_in_maps(query, key, value, mask, Wq, Wk, Wv), trace=True)
    out = np.stack([res.results[b]["out_o"] for b in range(B)])
    score = np.stack([res.results[b]["score_o"] for b in range(B)])
    return (out, score), res


# revision 20
# speedup vs baseline: 1.0291x; 1.0291x over previous
"""Trainium2 Bass kernel for single-head attention (nn_Attention_49881750176038).

reference computes, per batch element b (B=8, S=2048, Dm=1024, Dk=512):
    q = query @ Wq ; k = key @ Wk ; v = value @ Wv          # [S, Dk]
    pre = (q @ k.T) / sqrt(Dk) + mask * (-1e9)              # [S, S]
    score = softmax(pre, axis=-1)
    out = score @ v                                          # [S, Dk]
returns (out, score).

Sharding: pure data-parallel over batch — core c owns batch element c.

Per-core plan (all matmuls fp16 operands, fp32 PSUM accumulation — fp16
products are exact in fp32, so matmul error is just the fp16 input
quantization ~2.4e-4):
  phase 1: DMA-transpose-load q^T/k^T/v^T (fp16, host-cast), project:
      Q^T[dk, q], K^T[dk, s] (stationary = W tiles), V[s, dk]
      (stationary = v^T tiles).
  phase 2 (per 128-row q-tile):
      S chunks in PSUM via Q^T.T @ K^T; mask folded in via an
      identity-matmul add (host pre-scales mask by -60000 in fp16;
      -60000/sqrt(512) drives exp to exactly 0, matching the reference's
      -1e9 additive mask for binary masks);
      exp on ScalarE with scale=1/sqrt(Dk), accum_out = row-sum (free
      softmax denominator); reciprocal on VectorE;
      score rows = exp * recip -> fp32 -> HBM;
      DMA-transpose exp(S) row-tile (SBUF->SBUF, 3D dest) for the
      P @ V matmul; normalize out rows by recip -> HBM.
No collectives needed.
"""
import sys

sys.path.insert(0, "/opt/trn_rl_repo")

import numpy as np
from contextlib import ExitStack

import concourse.bacc as bacc
import concourse.tile as tile
from concourse import mybir
from concourse.bass_utils import run_bass_kernel_spmd

B, S, DM, DK, P = 8, 2048, 1024, 512, 128
NT_S = S // P      # 16 q/s tiles
NT_DM = DM // P    # 8 dm tiles
NT_DK = DK // P    # 4 dk tiles
NCH = S // 512     # 4 s-chunks of 512
CH = 512
SCALE = 1.0 / float(np.sqrt(np.float32(DK)))
MASKC = -60000.0   # fp16-representable; * SCALE -> -2652 -> exp -> 0.0

f16 = mybir.dt.float16
f32 = mybir.dt.float32

_NC = None


def _build():
    nc = bacc.Bacc("TRN2", target_bir_lowering=False, debug=False, num_devices=B)

    q16 = nc.dram_tensor("q16", [S, DM], f16, kind="ExternalInput").ap()
    k16 = nc.dram_tensor("k16", [S, DM], f16, kind="ExternalInput").ap()
    v16 = nc.dram_tensor("v16", [S, DM], f16, kind="ExternalInput").ap()
    m16 = nc.dram_tensor("m16", [S, S], f16, kind="ExternalInput").ap()
    wq16 = nc.dram_tensor("wq16", [DM, DK], f16, kind="ExternalInput").ap()
    wk16 = nc.dram_tensor("wk16", [DM, DK], f16, kind="ExternalInput").ap()
    wv16 = nc.dram_tensor("wv16", [DM, DK], f16, kind="ExternalInput").ap()

    score_o = nc.dram_tensor("score_o", [S, S], f32, kind="ExternalOutput").ap()
    out_o = nc.dram_tensor("out_o", [S, DK], f32, kind="ExternalOutput").ap()

    ident_np = np.eye(P, dtype=np.float16)
    ident_dram = nc.inline_tensor(ident_np, name="ident").ap()

    with tile.TileContext(nc) as tc:
        with ExitStack() as ctx:
            consts = ctx.enter_context(tc.tile_pool(name="consts", bufs=1))
            ps_pool = ctx.enter_context(tc.tile_pool(name="ps", bufs=7, space="PSUM"))
            pv_pool = ctx.enter_context(tc.tile_pool(name="pv", bufs=1, space="PSUM"))

            ident = consts.tile([P, P], f16)
            nc.sync.dma_start(out=ident, in_=ident_dram)
            # HAM warmup: keep the PE busy during the initial transpose fill
            # so the clock gate opens before real matmuls arrive.
            warm = ps_pool.tile([P, CH], f32, tag="ps")
            for i in range(48):
                nc.tensor.matmul(
                    warm[:, 0:P], ident, ident, start=(i == 0), stop=(i == 47)
                )
            warm_sink = consts.tile([P, 1], f32)
            nc.scalar.copy(out=warm_sink, in_=warm[:, 0:1])
            QT = consts.tile([P, NT_DK, S], f16)  # Q^T: [dk_in_tile, dk_tile, q]
            KT = consts.tile([P, NT_DK, S], f16)
            V = consts.tile([P, NT_S, DK], f16)   # V: [s_in_tile, s_tile, dk]

            # ---- phase 1: transposed loads + projections (scoped pools,
            # freed before phase 2 opens its pools)
            with ExitStack() as p1:
                w_pool = p1.enter_context(tc.tile_pool(name="w", bufs=1))
                xt_pool = p1.enter_context(tc.tile_pool(name="xt", bufs=18))

                wq_sb = w_pool.tile([P, NT_DM, DK], f16)
                wk_sb = w_pool.tile([P, NT_DM, DK], f16)
                wv_sb = w_pool.tile([P, NT_DM, DK], f16)
                for wsb, wap in ((wq_sb, wq16), (wk_sb, wk16), (wv_sb, wv16)):
                    nc.sync.dma_start(
                        out=wsb, in_=wap.rearrange("(t p) d -> p t d", p=P)
                    )

                def load_xT(xap):
                    ts = []
                    for ci in range(NT_DM):
                        t = xt_pool.tile([P, S], f16, tag="xt")
                        nc.sync.dma_start(
                            out=t, in_=xap[:, ci * P : (ci + 1) * P], transpose=True
                        )
                        ts.append(t)
                    return ts

                def project(xts, wsb, dst):
                    for qc in range(NCH):
                        for dki in range(NT_DK):
                            ps = ps_pool.tile([P, CH], f32, tag="ps")
                            for dmi in range(NT_DM):
                                nc.tensor.matmul(
                                    ps,
                                    wsb[:, dmi, dki * P : (dki + 1) * P],
                                    xts[dmi][:, qc * CH : (qc + 1) * CH],
                                    start=(dmi == 0),
                                    stop=(dmi == NT_DM - 1),
                                )
                            nc.scalar.copy(
                                out=dst[:, dki, qc * CH : (qc + 1) * CH], in_=ps
                            )

                kts = load_xT(k16)
                qts = load_xT(q16)
                project(kts, wk_sb, KT)
                vts = load_xT(v16)
                project(qts, wq_sb, QT)
                for si in range(NT_S):
                    ps = ps_pool.tile([P, DK], f32, tag="ps")
                    for dmi in range(NT_DM):
                        nc.tensor.matmul(
                            ps,
                            vts[dmi][:, si * P : (si + 1) * P],
                            wv_sb[:, dmi, :],
                            start=(dmi == 0),
                            stop=(dmi == NT_DM - 1),
                        )
                    nc.vector.tensor_copy(V[:, si, :], ps)

            # ---- phase 2 pools (open after phase-1 SBUF is released)
            mask_pool = ctx.enter_context(tc.tile_pool(name="mask", bufs=6))
            exp_pool = ctx.enter_context(tc.tile_pool(name="exp", bufs=4))
            expt_pool = ctx.enter_context(tc.tile_pool(name="expt", bufs=6))
            score_pool = ctx.enter_context(tc.tile_pool(name="score", bufs=3))
            outs_pool = ctx.enter_context(tc.tile_pool(name="outs", bufs=3))
            den_pool = ctx.enter_context(tc.tile_pool(name="den", bufs=6))

            # ---- phase 2: attention per q-tile.
            # P@V is software-pipelined two iterations behind S so the PE
            # never waits on the exp -> transpose chain (and stays HAM-warm).
            def emit_pv(ett, rec, qi):
                po = pv_pool.tile([P, DK], f32, tag="po")
                for si in range(NT_S):
                    nc.tensor.matmul(
                        po,
                        ett[:, si, :],
                        V[:, si, :],
                        start=(si == 0),
                        stop=(si == NT_S - 1),
                    )
                oS = outs_pool.tile([P, DK], f32)
                nc.vector.tensor_scalar_mul(oS, po, rec)
                nc.gpsimd.dma_start(out=out_o[qi * P : (qi + 1) * P, :], in_=oS)

            # mask prefetch (distance 3) so loads never sit behind
            # compute-dependent stores in the ring FIFO
            masks = {}

            def load_mask(qj):
                mt = mask_pool.tile([P, S], f16)
                nc.sync.dma_start(out=mt, in_=m16[qj * P : (qj + 1) * P, :])
                masks[qj] = mt

            for qj in range(3):
                load_mask(qj)

            pending = []
            for qi in range(NT_S):
                if qi + 3 < NT_S:
                    load_mask(qi + 3)
                mt = masks.pop(qi)

                et = exp_pool.tile([P, S], f16)
                den4 = den_pool.tile([P, NCH], f32, tag="den4")
                for sc in range(NCH):
                    ps = ps_pool.tile([P, CH], f32, tag="ps")
                    for dki in range(NT_DK):
                        nc.tensor.matmul(
                            ps,
                            QT[:, dki, qi * P : (qi + 1) * P],
                            KT[:, dki, sc * CH : (sc + 1) * CH],
                            start=(dki == 0),
                            stop=(dki == NT_DK - 1),
                        )
                    nc.vector.tensor_add(ps, ps, mt[:, sc * CH : (sc + 1) * CH])
                    nc.scalar.activation(
                        out=et[:, sc * CH : (sc + 1) * CH],
                        in_=ps,
                        func=mybir.ActivationFunctionType.Exp,
                        scale=SCALE,
                        accum_out=den4[:, sc : sc + 1],
                    )

                den = den_pool.tile([P, 1], f32, tag="den")
                nc.vector.reduce_sum(den, den4, axis=mybir.AxisListType.X)
                rec = den_pool.tile([P, 1], f32, tag="rec")
                nc.vector.reciprocal(rec, den)

                ett = expt_pool.tile([P, NT_S, P], f16)
                nc.sync.dma_start(out=ett, in_=et, transpose=True)

                sc32 = score_pool.tile([P, S], f32)
                nc.vector.tensor_scalar_mul(sc32, et, rec)
                nc.gpsimd.dma_start(
                    out=score_o[qi * P : (qi + 1) * P, :], in_=sc32
                )

                pending.append((ett, rec, qi))
                lag = 3 if qi < NT_S - 3 else NT_S - 1 - qi
                while len(pending) > lag:
                    emit_pv(*pending.pop(0))
            for args in pending:
                emit_pv(*args)

    nc.compile()
    return nc


def _get_nc():
    global _NC
    if _NC is None:
        _NC = _build()
    return _NC


def _make_in_maps(query, key, value, mask, Wq, Wk, Wv):
    query = np.asarray(query, dtype=np.float32)
    key = np.asarray(key, dtype=np.float32)
    value = np.asarray(value, dtype=np.float32)
    mask = np.asarray(mask, dtype=np.float32)
    wq = np.asarray(Wq, dtype=np.float32).astype(np.float16)
    wk = np.asarray(Wk, dtype=np.float32).astype(np.float16)
    wv = np.asarray(Wv, dtype=np.float32).astype(np.float16)
    in_maps = []
    for b in range(B):
        in_maps.append(
            {
                "q16": query[b].astype(np.float16),
                "k16": key[b].astype(np.float16),
                "v16": value[b].astype(np.float16),
                "m16": (mask[b] * np.float32(MASKC)).astype(np.float16),
                "wq16": wq,
                "wk16": wk,
                "wv16": wv,
            }
        )
    return in_maps


def _run(in_maps, trace=False, trace_kwargs=None):
    nc = _get_nc()
    return run_bass_kernel_spmd(
        nc,
        in_maps,
        list(range(B)),
        trace=trace,
        **(trace_kwargs or {}),
    )


def kernel(query, key, value, mask, Wq, Wk, Wv):
    res = _run(_make_in_maps(query, key, value, mask, Wq, Wk, Wv))
    out = np.stack([res.results[b]["out_o"] for b in range(B)])
    score = np.stack([res.results[b]["score_o"] for b in range(B)])
    return out, score


def kernel_traced(query, key, value, mask, Wq, Wk, Wv):
    """Same as kernel() but with NTFF profiling; returns ((out, score), results)."""
    res = _run(_make_in_maps(query, key, value, mask, Wq, Wk, Wv), trace=True)
    out = np.stack([res.results[b]["out_o"] for b in range(B)])
    score = np.stack([res.results[b]["score_o"] for b in range(B)])
    return (out, score), res
